# revision 9
# baseline (speedup 1.0000x reference)
"""Trainium2 Bass kernel for CartNN minimal-NEAT forward pass.

Computes out = tanh(tanh(x @ w + b))[:, None] for x [16384, 4096] f32,
w [4096] f32, b [1] f32, data-parallel across 8 NeuronCores (2048 batch
rows per core).

Memory-bound. The f32 stream floor is ~94 us/core (358 GB/s HBM per
core); the tolerance (2e-2) leaves ~10x headroom over fp16 rounding
(~2.3e-3 measured end-to-end on the real data), so x and w are cast to
fp16 on the host and streamed as 16 MiB/core (~43 us measured stream).

Engine facts measured on HW (NTFF traces of this problem):
  - DVE tensor_mul fp16 hits 2x_1p: ~2.2 us / [128,4096] tile.
  - DVE affine_mul_reduce (fused mult+reduce custom op): 1x only,
    ~4.4 us/tile regardless of dtype.
  - DVE tensor_scalar-with-accum claims 2x/4x modes in the cost model
    but executes at 1x (~4.4 us) on HW -- mult+that reduce (6.6 us/tile)
    loses to the fused op; do not use.
  - ScalarE activation(Copy, accum_out): ~3.6 us/tile free-dim reduce.

Schedule: DVE multiplies the ScalarE-reduced tiles (2.2 us) and runs the
fused op for tiles {3,6,9,12,15} (4.4 us); ScalarE reduces the other 11
tiles. Tile 0's mults are split along K so ScalarE's reduce chain starts
~13 us (w broadcast chunks gate the first quarters); tile 15 is fused in
K-quarters (loads and compute) so the last piece only waits on the last
256 KiB of the stream. Product pool has 6 buffers so DVE never stalls on
a slot while ScalarE drains its backlog (~5-tile peak).

Layout/DMA: x streams as 16 [128, 4096] fp16 tiles on the sync HWDGE
ring, which carries nothing else; w (8 KiB) and b ride the scalar ring
and TensorE outer-products broadcast w to all partitions (ScalarE copies
PSUM->SBUF casting f32->fp16, exact). Split loads are kept to tiles
0/15: each dma_start's completion costs extra occupancy on the last DMA
engine, which straggles if the kernel issues too many small DMAs.
Output: tanh(tanh(acc + b)) on ScalarE, TensorE transpose [128,16] ->
[16,128], one 8 KiB DMA of 512B-contiguous rows from the scalar ring.
"""

import numpy as np

import concourse.bacc as bacc
import concourse.mybir as mybir
from concourse.bass_utils import run_bass_kernel_spmd
from concourse.masks import make_identity
from concourse.tile import TileContext

N_CORES = 8
BATCH = 16384
IN_SIZE = 4096
P = 128
B_PER_CORE = BATCH // N_CORES  # 2048
N_TILES = B_PER_CORE // P  # 16

FUSED_TILES = (3, 6, 9, 12)  # + t15 (quartered); rest reduce on ScalarE

_NC_CACHE = None


def _build():
    nc = bacc.Bacc(
        "TRN2",
        target_bir_lowering=False,
        debug=False,
        num_devices=N_CORES,
    )
    x = nc.dram_tensor(
        "x", [B_PER_CORE, IN_SIZE], mybir.dt.float16, kind="ExternalInput"
    )
    w = nc.dram_tensor("w", [IN_SIZE], mybir.dt.float16, kind="ExternalInput")
    b = nc.dram_tensor("b", [1], mybir.dt.float32, kind="ExternalInput")
    y = nc.dram_tensor("y", [B_PER_CORE, 1], mybir.dt.float32, kind="ExternalOutput")

    xt = x.rearrange("(t p) k -> t p k", p=P)  # [16, 128, 4096]
    yT = y.rearrange("(t p) o -> t (p o)", p=P)  # [16, 128], 512B rows

    f16 = mybir.dt.float16
    f32 = mybir.dt.float32

    with TileContext(nc) as tc:
        with (
            tc.tile_pool(name="xpool", bufs=13) as xpool,
            tc.tile_pool(name="prods", bufs=6) as prpool,
            tc.tile_pool(name="scratch", bufs=1) as spool,
            tc.tile_pool(name="consts", bufs=1) as cpool,
            tc.tile_pool(name="psum", bufs=1, space="PSUM") as ppool,
        ):
            # w (8 KiB) and b ride the scalar ring so the sync ring carries
            # nothing but the x stream.
            w_1K = cpool.tile([1, IN_SIZE], f16)
            nc.scalar.dma_start(out=w_1K[:], in_=w[None, :])
            b_11 = cpool.tile([1, 1], f32)
            nc.scalar.dma_start(out=b_11[:], in_=b[None, :])
            # Memsets on GpSimd: its sequencer comes up ~3 us before DVE's,
            # so the PE w-broadcast (which waits on ones) starts earlier and
            # the ScalarE reduce chain (gated on the first products) moves
            # up by the same amount.
            ones_1P = cpool.tile([1, P], f16)
            nc.gpsimd.memset(ones_1P[:], 1.0)
            ones_1P_f32 = cpool.tile([1, P], f32)
            nc.gpsimd.memset(ones_1P_f32[:], 1.0)
            w_PK = cpool.tile([P, IN_SIZE], f16)
            NCHUNK = 512
            for c in range(IN_SIZE // NCHUNK):
                cs = slice(c * NCHUNK, (c + 1) * NCHUNK)
                w_psum = ppool.tile([P, NCHUNK], f32, bufs=2)
                nc.tensor.matmul(w_psum[:], ones_1P[:], w_1K[0:1, cs])
                nc.scalar.copy(w_PK[:, cs], w_psum[:])
            b_psum = ppool.tile([P, 1], f32)
            nc.tensor.matmul(b_psum[:], ones_1P_f32[:], b_11[:])
            b_P1 = cpool.tile([P, 1], f32)
            nc.scalar.copy(b_P1[:], b_psum[:])
            ident = cpool.tile([P, P], f32)
            make_identity(nc, ident[:])

            acc_PT = cpool.tile([P, N_TILES], f32)
            acc_sp = cpool.tile([P, 3], f32)  # spill slots for t15 quarters
            prod_fused = spool.tile([P, IN_SIZE], f16, name="prod_fused")
            x_tiles = {}

            def load_x(t, nsplit=1):
                x_PK = xpool.tile([P, IN_SIZE], f16)
                kq = IN_SIZE // nsplit
                for s in range(nsplit):
                    seg = slice(s * kq, (s + 1) * kq)
                    nc.sync.dma_start(out=x_PK[:, seg], in_=xt[t][:, seg])
                x_tiles[t] = x_PK

            def emit_fused(t, seg, acc):
                nc.vector.affine_mul_reduce(
                    out=prod_fused[:, seg],
                    accum_out=acc,
                    in0=x_tiles[t][:, seg],
                    in1=w_PK[:, seg],
                    scale=1.0,
                    bias=0.0,
                )

            def scalar_reduce(src, acc):
                nc.scalar.activation(
                    src[:],
                    src[:],
                    mybir.ActivationFunctionType.Copy,
                    accum_out=acc,
                )

            FULL = slice(0, IN_SIZE)
            KQ = IN_SIZE // 4

            # Tile 0: mult in quarters so DVE starts as soon as the first w
            # chunks are broadcast; ScalarE reduces the full product once.
            load_x(0, nsplit=4)
            prod0 = prpool.tile([P, IN_SIZE], f16, name="prod0", bufs=1)
            for s in range(4):
                seg = slice(s * KQ, (s + 1) * KQ)
                nc.vector.tensor_mul(prod0[:, seg], x_tiles[0][:, seg], w_PK[:, seg])
            scalar_reduce(prod0, acc_PT[:, 0:1])

            # Tiles 1..14: fused on DVE or mult + ScalarE reduce.
            for t in range(1, N_TILES - 1):
                load_x(t)
                if t in FUSED_TILES:
                    emit_fused(t, FULL, acc_PT[:, t : t + 1])
                else:
                    prod = prpool.tile([P, IN_SIZE], f16)
                    nc.vector.tensor_mul(prod[:], x_tiles[t][:], w_PK[:])
                    scalar_reduce(prod, acc_PT[:, t : t + 1])

            # t15: quarters (loads AND fused compute) to shrink the tail.
            t15 = N_TILES - 1
            load_x(t15, nsplit=4)
            for s in range(4):
                seg = slice(s * KQ, (s + 1) * KQ)
                acc = acc_PT[:, t15 : t15 + 1] if s == 0 else acc_sp[:, s - 1 : s]
                emit_fused(t15, seg, acc)

            # Combine t15's quarter partial sums.
            nc.vector.tensor_add(acc_sp[:, 0:1], acc_sp[:, 0:1], acc_sp[:, 1:2])
            nc.vector.tensor_add(
                acc_PT[:, t15 : t15 + 1], acc_PT[:, t15 : t15 + 1], acc_sp[:, 2:3]
            )
            nc.vector.tensor_add(
                acc_PT[:, t15 : t15 + 1], acc_PT[:, t15 : t15 + 1], acc_sp[:, 0:1]
            )

            # Output path: tanh(tanh(acc + b)) on ScalarE, TensorE
            # transpose [128, 16] -> [16, 128] so the output DMA writes
            # 512B-contiguous runs, DMA from the scalar ring.
            y_PT = cpool.tile([P, N_TILES], f32)
            nc.scalar.activation(
                y_PT[:],
                acc_PT[:],
                mybir.ActivationFunctionType.Tanh,
                bias=b_P1[:],
            )
            nc.scalar.activation(y_PT[:], y_PT[:], mybir.ActivationFunctionType.Tanh)
            y_psum = ppool.tile([N_TILES, P], f32)
            nc.tensor.transpose(y_psum[:], y_PT[:], ident[:])
            y_TP = cpool.tile([N_TILES, P], f32)
            nc.scalar.copy(y_TP[:], y_psum[:])
            nc.scalar.dma_start(out=yT, in_=y_TP[:])
    nc.compile()
    return nc


def _get_nc():
    global _NC_CACHE
    if _NC_CACHE is None:
        _NC_CACHE = _build()
    return _NC_CACHE


def _run(x, w, b, **spmd_kwargs):
    """Shard, execute on 8 cores, gather. Returns (out, BassKernelResults)."""
    x = np.ascontiguousarray(np.asarray(x, dtype=np.float32).astype(np.float16))
    w = np.ascontiguousarray(np.asarray(w, dtype=np.float32).astype(np.float16))
    b = np.ascontiguousarray(np.asarray(b, dtype=np.float32))
    assert x.shape == (BATCH, IN_SIZE), x.shape

    nc = _get_nc()
    in_maps = [
        {"x": x[c * B_PER_CORE : (c + 1) * B_PER_CORE], "w": w, "b": b}
        for c in range(N_CORES)
    ]
    res = run_bass_kernel_spmd(nc, in_maps, list(range(N_CORES)), **spmd_kwargs)
    out = np.concatenate(
        [np.asarray(res.results[c]["y"]) for c in range(N_CORES)], axis=0
    )
    return out.astype(np.float32, copy=False), res


def kernel(x, w, b):
    try:
        out, _ = _run(x, w, b)
    except Exception:
        # Transient device-wedge (NRT_EXEC_UNIT_UNRECOVERABLE) has been
        # observed once on a first run and succeeded on retry.
        out, _ = _run(x, w, b)
    return out


# revision 10
# speedup vs baseline: 1.0393x; 1.0393x over previous
"""Trainium2 Bass kernel for CartNN minimal-NEAT forward pass.

Computes out = tanh(tanh(x @ w + b))[:, None] for x [16384, 4096] f32,
w [4096] f32, b [1] f32, data-parallel across 8 NeuronCores (2048 batch
rows per core).

Memory-bound. The f32 stream floor is ~94 us/core (358 GB/s HBM per
core); the tolerance (2e-2) leaves ~10x headroom over fp16 rounding
(~2.3e-3 measured end-to-end on the real data), so x and w are cast to
fp16 on the host and streamed as 16 MiB/core (~43 us measured stream).

Engine facts measured on HW (NTFF traces of this problem):
  - DVE tensor_mul fp16 hits 2x_1p: ~2.2 us / [128,4096] tile.
  - DVE affine_mul_reduce (fused mult+reduce custom op): 1x only,
    ~4.4 us/tile regardless of dtype.
  - DVE tensor_scalar-with-accum claims 2x/4x modes in the cost model
    but executes at 1x (~4.4 us) on HW -- mult+that reduce (6.6 us/tile)
    loses to the fused op; do not use.
  - ScalarE activation(Copy, accum_out): ~3.6 us/tile free-dim reduce.

Schedule: DVE multiplies the ScalarE-reduced tiles (2.2 us) and runs the
fused op for tiles {3,6,9,12,15} (4.4 us); ScalarE reduces the other 11
tiles. Tile 0's mults are split along K so ScalarE's reduce chain starts
~13 us (w broadcast chunks gate the first quarters); tile 15 is fused in
K-quarters (loads and compute) so the last piece only waits on the last
256 KiB of the stream. Product pool has 6 buffers so DVE never stalls on
a slot while ScalarE drains its backlog (~5-tile peak).

Layout/DMA: x streams as 16 [128, 4096] fp16 tiles on the sync HWDGE
ring, which carries nothing else; w (8 KiB) and b ride the scalar ring
and TensorE outer-products broadcast w to all partitions (ScalarE copies
PSUM->SBUF casting f32->fp16, exact). Split loads are kept to tiles
0/15: each dma_start's completion costs extra occupancy on the last DMA
engine, which straggles if the kernel issues too many small DMAs.
Output: tanh(tanh(acc + b)) on ScalarE, TensorE transpose [128,16] ->
[16,128], one 8 KiB DMA of 512B-contiguous rows from the scalar ring.
"""

import numpy as np

import concourse.bacc as bacc
import concourse.mybir as mybir
from concourse.bass_utils import run_bass_kernel_spmd
from concourse.masks import make_identity
from concourse.tile import TileContext

N_CORES = 8
BATCH = 16384
IN_SIZE = 4096
P = 128
B_PER_CORE = BATCH // N_CORES  # 2048
N_TILES = B_PER_CORE // P  # 16

FUSED_TILES = (3, 6, 9, 12)  # + t15 (quartered); rest reduce on ScalarE

_NC_CACHE = None


def _build():
    nc = bacc.Bacc(
        "TRN2",
        target_bir_lowering=False,
        debug=False,
        num_devices=N_CORES,
    )
    x = nc.dram_tensor(
        "x", [B_PER_CORE, IN_SIZE], mybir.dt.float16, kind="ExternalInput"
    )
    w = nc.dram_tensor("w", [IN_SIZE], mybir.dt.float16, kind="ExternalInput")
    b = nc.dram_tensor("b", [1], mybir.dt.float32, kind="ExternalInput")
    y = nc.dram_tensor("y", [B_PER_CORE, 1], mybir.dt.float32, kind="ExternalOutput")

    xt = x.rearrange("(t p) k -> t p k", p=P)  # [16, 128, 4096]
    yT = y.rearrange("(t p) o -> t (p o)", p=P)  # [16, 128], 512B rows

    f16 = mybir.dt.float16
    f32 = mybir.dt.float32

    with TileContext(nc) as tc:
        with (
            tc.tile_pool(name="xpool", bufs=13) as xpool,
            tc.tile_pool(name="prods", bufs=6) as prpool,
            tc.tile_pool(name="scratch", bufs=1) as spool,
            tc.tile_pool(name="consts", bufs=1) as cpool,
            tc.tile_pool(name="psum", bufs=1, space="PSUM") as ppool,
        ):
            # w (8 KiB) and b ride the scalar ring so the sync ring carries
            # nothing but the x stream.
            w_1K = cpool.tile([1, IN_SIZE], f16)
            nc.scalar.dma_start(out=w_1K[:], in_=w[None, :])
            b_11 = cpool.tile([1, 1], f32)
            nc.scalar.dma_start(out=b_11[:], in_=b[None, :])
            # Memsets stay on DVE: moving them to GpSimd (earlier sequencer
            # start) was tried and measured SLOWER (71.8 vs 68.1 us) -- the
            # Q7 launch path delays the PE w-broadcast instead of helping.
            ones_1P = cpool.tile([1, P], f16)
            nc.vector.memset(ones_1P[:], 1.0)
            ones_1P_f32 = cpool.tile([1, P], f32)
            nc.vector.memset(ones_1P_f32[:], 1.0)
            w_PK = cpool.tile([P, IN_SIZE], f16)
            NCHUNK = 512
            for c in range(IN_SIZE // NCHUNK):
                cs = slice(c * NCHUNK, (c + 1) * NCHUNK)
                w_psum = ppool.tile([P, NCHUNK], f32, bufs=2)
                nc.tensor.matmul(w_psum[:], ones_1P[:], w_1K[0:1, cs])
                nc.scalar.copy(w_PK[:, cs], w_psum[:])
            b_psum = ppool.tile([P, 1], f32)
            nc.tensor.matmul(b_psum[:], ones_1P_f32[:], b_11[:])
            b_P1 = cpool.tile([P, 1], f32)
            nc.scalar.copy(b_P1[:], b_psum[:])
            ident = cpool.tile([P, P], f32)
            make_identity(nc, ident[:])

            acc_PT = cpool.tile([P, N_TILES], f32)
            acc_sp = cpool.tile([P, 3], f32)  # spill slots for t15 quarters
            prod_fused = spool.tile([P, IN_SIZE], f16, name="prod_fused")
            x_tiles = {}

            def load_x(t, nsplit=1):
                x_PK = xpool.tile([P, IN_SIZE], f16)
                kq = IN_SIZE // nsplit
                for s in range(nsplit):
                    seg = slice(s * kq, (s + 1) * kq)
                    nc.sync.dma_start(out=x_PK[:, seg], in_=xt[t][:, seg])
                x_tiles[t] = x_PK

            def emit_fused(t, seg, acc):
                nc.vector.affine_mul_reduce(
                    out=prod_fused[:, seg],
                    accum_out=acc,
                    in0=x_tiles[t][:, seg],
                    in1=w_PK[:, seg],
                    scale=1.0,
                    bias=0.0,
                )

            def scalar_reduce(src, acc):
                nc.scalar.activation(
                    src[:],
                    src[:],
                    mybir.ActivationFunctionType.Copy,
                    accum_out=acc,
                )

            FULL = slice(0, IN_SIZE)
            KQ = IN_SIZE // 4

            # Tile 0: mult in quarters so DVE starts as soon as the first w
            # chunks are broadcast; ScalarE reduces the full product once.
            load_x(0, nsplit=4)
            prod0 = prpool.tile([P, IN_SIZE], f16, name="prod0", bufs=1)
            for s in range(4):
                seg = slice(s * KQ, (s + 1) * KQ)
                nc.vector.tensor_mul(prod0[:, seg], x_tiles[0][:, seg], w_PK[:, seg])
            scalar_reduce(prod0, acc_PT[:, 0:1])

            # Tiles 1..14: fused on DVE or mult + ScalarE reduce.
            for t in range(1, N_TILES - 1):
                load_x(t)
                if t in FUSED_TILES:
                    emit_fused(t, FULL, acc_PT[:, t : t + 1])
                else:
                    prod = prpool.tile([P, IN_SIZE], f16)
                    nc.vector.tensor_mul(prod[:], x_tiles[t][:], w_PK[:])
                    scalar_reduce(prod, acc_PT[:, t : t + 1])

            # t15: quarters (loads AND fused compute) to shrink the tail.
            t15 = N_TILES - 1
            load_x(t15, nsplit=4)
            for s in range(4):
                seg = slice(s * KQ, (s + 1) * KQ)
                acc = acc_PT[:, t15 : t15 + 1] if s == 0 else acc_sp[:, s - 1 : s]
                emit_fused(t15, seg, acc)

            # Combine t15's quarter partial sums.
            nc.vector.tensor_add(acc_sp[:, 0:1], acc_sp[:, 0:1], acc_sp[:, 1:2])
            nc.vector.tensor_add(
                acc_PT[:, t15 : t15 + 1], acc_PT[:, t15 : t15 + 1], acc_sp[:, 2:3]
            )
            nc.vector.tensor_add(
                acc_PT[:, t15 : t15 + 1], acc_PT[:, t15 : t15 + 1], acc_sp[:, 0:1]
            )

            # Output path: tanh(tanh(acc + b)) on ScalarE, TensorE
            # transpose [128, 16] -> [16, 128] so the output DMA writes
            # 512B-contiguous runs, DMA from the scalar ring.
            y_PT = cpool.tile([P, N_TILES], f32)
            nc.scalar.activation(
                y_PT[:],
                acc_PT[:],
                mybir.ActivationFunctionType.Tanh,
                bias=b_P1[:],
            )
            nc.scalar.activation(y_PT[:], y_PT[:], mybir.ActivationFunctionType.Tanh)
            y_psum = ppool.tile([N_TILES, P], f32)
            nc.tensor.transpose(y_psum[:], y_PT[:], ident[:])
            y_TP = cpool.tile([N_TILES, P], f32)
            nc.scalar.copy(y_TP[:], y_psum[:])
            nc.scalar.dma_start(out=yT, in_=y_TP[:])
    nc.compile()
    return nc


def _get_nc():
    global _NC_CACHE
    if _NC_CACHE is None:
        _NC_CACHE = _build()
    return _NC_CACHE


def _run(x, w, b, **spmd_kwargs):
    """Shard, execute on 8 cores, gather. Returns (out, BassKernelResults)."""
    x = np.ascontiguousarray(np.asarray(x, dtype=np.float32).astype(np.float16))
    w = np.ascontiguousarray(np.asarray(w, dtype=np.float32).astype(np.float16))
    b = np.ascontiguousarray(np.asarray(b, dtype=np.float32))
    assert x.shape == (BATCH, IN_SIZE), x.shape

    nc = _get_nc()
    in_maps = [
        {"x": x[c * B_PER_CORE : (c + 1) * B_PER_CORE], "w": w, "b": b}
        for c in range(N_CORES)
    ]
    res = run_bass_kernel_spmd(nc, in_maps, list(range(N_CORES)), **spmd_kwargs)
    out = np.concatenate(
        [np.asarray(res.results[c]["y"]) for c in range(N_CORES)], axis=0
    )
    return out.astype(np.float32, copy=False), res


def kernel(x, w, b):
    try:
        out, _ = _run(x, w, b)
    except Exception:
        # Transient device-wedge (NRT_EXEC_UNIT_UNRECOVERABLE) has been
        # observed once on a first run and succeeded on retry.
        out, _ = _run(x, w, b)
    return out
